# revision 18
# baseline (speedup 1.0000x reference)
"""Trainium2 Bass kernel for sinusoidal positional encoding.

reference: x [2_000_000, 3] f32 -> out [2_000_000, 60] f32 with
  out[n, c*20 + s*10 + i] = sin(x[n,c] * 2^i)  if s == 0
                            cos(x[n,c] * 2^i)  if s == 1

Sharding: pure data-parallel over rows across 8 NeuronCores (250k rows each,
identical SPMD program; results concatenated).

Per-core pipeline, tiles of P=128 partitions x R rows/partition (R ramps
8->16->32->64 then steady 96 so output DMA starts early):
  DVE    : phase = x * 2^i     (broadcast-AP tensor_tensor)
           half of the magic-round (kt = phase*inv2pi + M, fused 2-imm TS)
           r = cody_waite(phase, k)  range-reduce to [-pi, pi]
           35% of |r| (sign-bit clear via i32 bitwise_and)
  GPSIMD : other half of magic-round, k = t - M, input DMAs (SWDGE)
  ACT    : 65% of |r| (Abs), out_sin = Sin(r),
           out_cos = Sin(|r|, scale=-1, bias=pi/2)   [= sin(pi/2-|r|) = cos(r)]
  DMA    : output written as quarter-tile (first 8 tiles) then half-tile
           DMAs alternating between the sync/scalar HWDGE rings (splits
           descriptor-gen across sequencers, starts the output stream early);
           first 14 tiles' input DMAs ride the otherwise-idle sync ring with
           a 10-deep prefetch pool, later inputs go via SWDGE so they never
           queue behind 1.5 MB output writes; a dummy Sin at program start
           hoists the ACT trig-table load off the first tile's critical path;
           the 2^i scale constants are built with 10 strided memsets instead
           of a DMA so no constant load gates the first tile

Work is column-split across engines so every engine's busy time (~154 us)
sits ~20 us under the ~175 us/core HBM roofline (63 MB I/O at ~358 GB/s).
The residual gap is pipeline warm-up. Cost-model (TimelineSim) span:
~190 us/core (1.085x roofline).
"""
import sys

if "/opt/trn_rl_repo" not in sys.path:
    sys.path.insert(0, "/opt/trn_rl_repo")

from contextlib import ExitStack

import numpy as np

import concourse.bacc as bacc
import concourse.mybir as mybir
import concourse.tile as tile
from concourse.bass_utils import run_bass_kernel_spmd

F32 = mybir.dt.float32
AF = mybir.ActivationFunctionType
ALU = mybir.AluOpType

N_CORES = 8
N_TOTAL = 2_000_000
NC_ROWS = N_TOTAL // N_CORES  # 250_000
D = 10
C = 3
OUT_COLS = 2 * D * C  # 60

PI = float(np.pi)
INV_2PI = float(np.float32(1.0 / (2.0 * np.pi)))
MAGIC = float(np.float32(1.5 * 2**23))

# 2*pi = C1 + C2 + C3 Cody-Waite split (C1/C2 have short mantissas so
# k*C1 and k*C2 are exact in f32 for the |k| <= ~500 seen here).
C1 = 6.28125
_d = np.float64(2 * np.pi) - np.float64(C1)
_c2 = np.float32(_d)
_c2 = np.frombuffer(
    np.uint32(np.frombuffer(_c2.tobytes(), dtype=np.uint32)[0] & 0xFFFFF000).tobytes(),
    dtype=np.float32,
)[0]
C2 = float(_c2)
C3 = float(np.float32(_d - np.float64(_c2)))

R_STEADY = 96
RAMP = (8, 16, 32, 64)
FM1_DVE = 0.50  # fraction of magic1 columns on DVE (rest GPSIMD)
FABS_ACT = 0.65  # fraction of abs columns on ACT (rest DVE)
N_IN_HWDGE = 14  # first N tiles' input DMAs ride the sync HWDGE ring (idle early)
N_QUARTER = 8  # first N tiles emit quarter-tile output DMAs (earlier DMA start)

TRACE = False  # set by test harness for NTFF profiling (native runs only)
LAST_RESULTS = None

_cached_nc = None


def _tiles():
    tiles = []
    n0 = 0
    for r in RAMP:
        tiles.append((n0, 128, r))
        n0 += 128 * r
    nfull = (NC_ROWS - n0) // (128 * R_STEADY)
    for _ in range(nfull):
        tiles.append((n0, 128, R_STEADY))
        n0 += 128 * R_STEADY
    rem = NC_ROWS - n0
    r_mid = rem // 128
    if r_mid:
        tiles.append((n0, 128, r_mid))
        n0 += 128 * r_mid
        rem -= 128 * r_mid
    if rem:
        tiles.append((n0, rem, 1))
        n0 += rem
    assert sum(p * r for _, p, r in tiles) == NC_ROWS
    return tiles


def _build_program():
    nc = bacc.Bacc("TRN2", target_bir_lowering=False, debug=False, num_devices=N_CORES)
    x = nc.dram_tensor("x", [NC_ROWS, C], F32, kind="ExternalInput").ap()
    out = nc.dram_tensor("out", [NC_ROWS, OUT_COLS], F32, kind="ExternalOutput").ap()

    with ExitStack() as ctx:
        tc = ctx.enter_context(tile.TileContext(nc))
        cpool = ctx.enter_context(tc.tile_pool(name="const", bufs=1))
        xpool = ctx.enter_context(tc.tile_pool(name="xin", bufs=10))
        ppool = ctx.enter_context(tc.tile_pool(name="ph", bufs=2))
        kpool = ctx.enter_context(tc.tile_pool(name="kk", bufs=2))
        rpool = ctx.enter_context(tc.tile_pool(name="rr", bufs=2))
        wpool = ctx.enter_context(tc.tile_pool(name="ww", bufs=2))
        opool = ctx.enter_context(tc.tile_pool(name="oo", bufs=3))

        # scale_t[p, c*10+i] = 2^i, built with 10 strided memsets (no DMA,
        # no ring traffic — the values are exact f32 powers of two)
        scale_t = cpool.tile([128, C * D], F32, tag="scale")
        sc3 = scale_t[:].rearrange("p (c i) -> p c i", i=D)
        for i_ in range(D):
            nc.gpsimd.memset(sc3[:, :, i_], float(2.0**i_))
        hpi_col = cpool.tile([128, 1], F32, tag="hpi")
        nc.gpsimd.memset(hpi_col[:], PI / 2)
        # dummy activation: hoists the trig table load off the critical path
        warm_t = cpool.tile([128, 1], F32, tag="warmt")
        nc.scalar.activation(warm_t[:], hpi_col[:], AF.Sin)

        for ti, (n0, P, R_) in enumerate(_tiles()):
            rows = P * R_
            M30 = R_ * C * D

            xt = xpool.tile([P, R_ * C], F32, tag="xt")
            in_eng = nc.sync if ti < N_IN_HWDGE else nc.gpsimd
            in_eng.dma_start(
                xt[:], x[n0 : n0 + rows, :].rearrange("(p r) c -> p (r c)", p=P)
            )

            # phase = x * 2^i, laid out [P, (r, c, i)]
            ph = ppool.tile([P, M30], F32, tag="ph")
            x_b = (
                xt[:]
                .rearrange("p (r c) -> p r c", c=C)
                .unsqueeze(3)
                .broadcast_to([P, R_, C, D])
            )
            sc_b = (
                scale_t[:P]
                .rearrange("p (c i) -> p c i", c=C)
                .unsqueeze(1)
                .broadcast_to([P, R_, C, D])
            )
            ph4 = ph[:].rearrange("p (r c i) -> p r c i", c=C, i=D)
            nc.vector.tensor_tensor(ph4, x_b, sc_b, ALU.mult)

            # k = round(phase / 2pi) via magic-number rounding, split DVE/GPSIMD
            kt = kpool.tile([P, M30], F32, tag="kt")
            s1 = (int(M30 * FM1_DVE) // 30) * 30
            if s1 > 0:
                nc.vector.tensor_scalar(
                    kt[:, :s1], ph[:, :s1], INV_2PI, MAGIC, ALU.mult, ALU.add
                )
            if s1 < M30:
                nc.gpsimd.tensor_scalar(
                    kt[:, s1:], ph[:, s1:], INV_2PI, MAGIC, ALU.mult, ALU.add
                )
            nc.gpsimd.tensor_scalar(kt[:], kt[:], MAGIC, None, ALU.subtract)

            # r = ((phase - k*C1) - k*C2) - k*C3  in [-pi, pi]
            rt = rpool.tile([P, M30], F32, tag="rt")
            nc.vector.cody_waite_cascade(rt[:], ph[:], kt[:], C1, C2, C3)

            # |r|, split ACT/DVE
            wt = wpool.tile([P, M30], F32, tag="wt")
            s2 = (int(M30 * FABS_ACT) // 30) * 30
            if s2 > 0:
                nc.scalar.activation(wt[:, :s2], rt[:, :s2], AF.Abs)
            if s2 < M30:
                # f32 abs = clear sign bit, on DVE (abs is not a valid Pool op)
                nc.vector.tensor_scalar(
                    wt[:, s2:].bitcast(mybir.dt.int32),
                    rt[:, s2:].bitcast(mybir.dt.int32),
                    0x7FFFFFFF,
                    None,
                    ALU.bitwise_and,
                )

            ot = opool.tile([P, R_ * OUT_COLS], F32, tag="ot")
            o5 = ot[:].rearrange("p (r c s i) -> p r c s i", c=C, s=2, i=D)
            r4 = rt[:].rearrange("p (r c i) -> p r c i", c=C, i=D)
            a4 = wt[:].rearrange("p (r c i) -> p r c i", c=C, i=D)
            out3 = out[n0 : n0 + rows, :].rearrange("(p r) c -> p r c", p=P)
            ot3 = ot[:].rearrange("p (r c) -> p r c", c=OUT_COLS)

            if ti < N_QUARTER and R_ >= 4:
                nsplit = 4
            elif R_ >= 2:
                nsplit = 2
            else:
                nsplit = 1
            step = R_ // nsplit
            halves = [
                (j * step, R_ if j == nsplit - 1 else (j + 1) * step)
                for j in range(nsplit)
            ]
            for h, (r_lo, r_hi) in enumerate(halves):
                nc.scalar.activation(
                    o5[:, r_lo:r_hi, :, 0, :], r4[:, r_lo:r_hi], AF.Sin
                )
                nc.scalar.activation(
                    o5[:, r_lo:r_hi, :, 1, :],
                    a4[:, r_lo:r_hi],
                    AF.Sin,
                    bias=hpi_col[:P],
                    scale=-1.0,
                )
                oeng = nc.sync if (ti + h) % 2 == 0 else nc.scalar
                oeng.dma_start(out3[:, r_lo:r_hi, :], ot3[:, r_lo:r_hi, :])

    nc.compile()
    return nc


def kernel(x: np.ndarray) -> np.ndarray:
    global _cached_nc, LAST_RESULTS
    x = np.ascontiguousarray(np.asarray(x, dtype=np.float32))
    assert x.shape == (N_TOTAL, C), x.shape

    if _cached_nc is None:
        _cached_nc = _build_program()
    nc = _cached_nc

    in_maps = [
        {"x": np.ascontiguousarray(x[i * NC_ROWS : (i + 1) * NC_ROWS])}
        for i in range(N_CORES)
    ]
    res = run_bass_kernel_spmd(nc, in_maps, core_ids=list(range(N_CORES)), trace=TRACE)
    out_full = np.concatenate([r["out"] for r in res.results], axis=0)
    # keep only timing metadata — retaining res.results would leak ~0.5 GB/call
    res.results = []
    LAST_RESULTS = res
    return out_full
